# revision 1
# baseline (speedup 1.0000x reference)
"""Longformer layer (B=2, S=4096, D=768, H=12, w=128, NG=32) on 8 TRN2 cores.

Sharding: sequence-parallel. Core c owns tokens [q0, q0+1024) of batch b=c//4,
q0 = (c%4)*1024. Each core computes band+global-key attention and the dense
pipeline for its 1024 tokens. Global-QUERY rows (tokens 0..NG of each batch)
need keys from every core, so each core also emits flash-style partials
(sum exp*v and sum exp over its local keys); the host combines those and runs
the dense tail for the 2*NG global rows during gather/unshard.

On-device layout is feature-major ("xT" = [feature, token]) so weights are the
stationary matmul operand. Band scores are computed TRANSPOSED ([key, query]):
the softmax denominator then comes from a ones-column folded into the AV
matmul, and no probs transpose is needed. Probs leave the ACT engine in bf16
for the AV matmuls. Softmax needs no max-subtraction: logits are O(+-2) here.
Band masking is a post-exp multiply by host-built 0/1 masks (validity is
static per chunk); the attention_mask enters as the per-partition exp bias.
"""

import math
import numpy as np
import ml_dtypes

import concourse.bacc as bacc
import concourse.mybir as mybir
import concourse.tile as tile
from concourse.masks import make_identity

F32 = mybir.dt.float32
F32R = mybir.dt.float32r
BF16 = mybir.dt.bfloat16
AF = mybir.ActivationFunctionType

B, S, D, FF = 2, 4096, 768, 3072
H, DH, W, NG = 12, 64, 128, 32
EPS = 1e-12
T = 1024           # owned tokens per core
TH = T + 2 * W     # with halo
KD = D // 128      # 6 feature tiles
KF = FF // 128     # 24
NCH = T // W       # 8 owned chunks
NJ = NCH + 2       # k-chunks j=-1..8  (jdx = j+1)
HALF = 512
N_CORES = 8
ISCALE = 1.0 / math.sqrt(DH)

_nc_cache = {}


def r(ap):
    return ap.bitcast(F32R)


def build_body(nc, tc, ap, ctx, sim_mode=False, stop_after=None):
    def dummy_out(pool, tiles, og_too=True):
        # fill outputs with defined data so a truncated build still runs
        z = pool.tile([128, T], F32, tag="zdum", name="zdum")
        nc.vector.memset(z, 0.0)
        for k in range(KD):
            nc.sync.dma_start(out=ap["outT"][k * 128:(k + 1) * 128, :], in_=z)
        if og_too:
            zg = pool.tile([65, NG], F32, tag="zgdum", name="zgdum")
            nc.vector.memset(zg, 1.0)
            for h in range(H):
                nc.sync.dma_start(out=ap["og"][h], in_=zg)
    gelu_f = AF.Identity if sim_mode else AF.Gelu
    persist = ctx.enter_context(tc.tile_pool(name="persist", bufs=1))

    # ---------------- constants / biases (small; rows load per-phase) -------
    ident = persist.tile([128, 128], F32, tag="ident", name="ident")
    make_identity(nc, ident)
    ones_col = persist.tile([128, 1], F32, tag="ones_col", name="ones_col")
    ones_row = persist.tile([1, HALF], F32, tag="ones_row", name="ones_row")
    ones_row64 = persist.tile([1, DH], F32, tag="ones_row64", name="ones_row64")
    for t_ in (ones_col, ones_row, ones_row64):
        nc.vector.memset(t_, 1.0)
        # rewrite through DVE so the value is "rounded to f32r" for matmuls
        nc.vector.tensor_scalar_mul(out=t_[:].bitcast(F32R), in0=t_, scalar1=1.0)
    eps_sc = persist.tile([1, 1], F32, tag="eps_sc", name="eps_sc")
    nc.vector.memset(eps_sc, EPS)

    def load_bias_cols(name, n):
        t = persist.tile([128, n], F32, tag=name)
        nc.sync.dma_start(out=t, in_=ap[name].rearrange("(k p) -> p k", p=128))
        return t

    bq_sb = load_bias_cols("bq", KD)
    bk_sb = load_bias_cols("bk", KD)
    bo_sb = load_bias_cols("bo", KD)
    bi_sb = load_bias_cols("bi", KF)
    bo2_sb = load_bias_cols("bo2", KD)

    bv_bc = persist.tile([128, D], F32, tag="bv_bc", name="bv_bc")
    nc.gpsimd.dma_start(out=bv_bc, in_=ap["bv"].unsqueeze(0).partition_broadcast(128))

    am_sb = persist.tile([128, NJ], F32, tag="am_sb", name="am_sb")
    nc.sync.dma_start(out=am_sb, in_=ap["am_halo"].rearrange("(k p) -> p k", p=128))
    amg_sb = persist.tile([NG, 1], F32, tag="amg_sb", name="amg_sb")
    nc.sync.dma_start(out=amg_sb, in_=ap["am_glob"].unsqueeze(1))

    def load_ln_rows(pool, gname, bname):
        g_row = pool.tile([1, D], F32R, tag="g_row", name="g_row")
        nc.sync.dma_start(out=g_row, in_=ap[gname].unsqueeze(0))
        b_row = pool.tile([1, D], F32, tag="b_row", name="b_row")
        nc.sync.dma_start(out=b_row, in_=ap[bname].unsqueeze(0))
        nb_row = pool.tile([1, D], F32, tag="nb_row", name="nb_row")
        nc.vector.tensor_scalar_mul(out=nb_row[:].bitcast(F32R), in0=b_row, scalar1=-1.0)
        return g_row, nb_row

    def layernorm(u_tiles, g_row, nb_row, dest_aps, ln_ps, ln_sb, row_ps):
        """u_tiles: KD SBUF tiles [128, HALF] (pre-norm, f32, feature-major).
        Writes LN(u)*g+b into dest_aps[k] ([128, HALF] APs)."""
        s1 = row_ps.tile([1, HALF], F32, tag="s1", name="s1")
        s2 = row_ps.tile([1, HALF], F32, tag="s2", name="s2")
        for k in range(KD):
            nc.tensor.matmul(s1[:], r(ones_col), r(u_tiles[k][:]),
                             start=(k == 0), stop=(k == KD - 1))
        for k in range(KD):
            usq = ln_sb.tile([128, HALF], F32, tag="usq", name="usq", bufs=2)
            nc.vector.tensor_mul(out=usq[:].bitcast(F32R), in0=u_tiles[k][:], in1=u_tiles[k][:])
            nc.tensor.matmul(s2[:], r(ones_col), r(usq[:]),
                             start=(k == 0), stop=(k == KD - 1))
        mu = ln_sb.tile([1, HALF], F32, tag="mu", name="mu")
        nc.vector.tensor_scalar_mul(out=mu, in0=s1, scalar1=1.0 / D)
        var = ln_sb.tile([1, HALF], F32, tag="var", name="var")
        nc.vector.tensor_scalar_mul(out=var, in0=s2, scalar1=1.0 / D)
        musq = ln_sb.tile([1, HALF], F32, tag="musq", name="musq")
        nc.vector.tensor_mul(out=musq, in0=mu, in1=mu)
        nc.vector.tensor_sub(out=var, in0=var, in1=musq)
        sd = ln_sb.tile([1, HALF], F32, tag="sd", name="sd")
        nc.scalar.activation(out=sd, in_=var, func=AF.Sqrt, bias=eps_sc[:])
        rstd = ln_sb.tile([1, HALF], F32, tag="rstd", name="rstd")
        with nc.allow_low_precision(reason="f32r rounding only"):
            nc.vector.reciprocal(out=rstd[:].bitcast(F32R), in_=sd)
        mr = ln_sb.tile([1, HALF], F32, tag="mr", name="mr")
        nc.vector.tensor_mul(out=mr[:].bitcast(F32R), in0=mu, in1=rstd)
        for k in range(KD):
            g1p = ln_ps.tile([128, HALF], F32, tag="g1p", name="g1p")
            nc.tensor.matmul(g1p[:], r(g_row[:, k * 128:(k + 1) * 128]), r(rstd[:]),
                             start=True, stop=True)
            g2p = ln_ps.tile([128, HALF], F32, tag="g2p", name="g2p")
            nc.tensor.matmul(g2p[:], r(g_row[:, k * 128:(k + 1) * 128]), r(mr[:]),
                             start=True, stop=False)
            nc.tensor.matmul(g2p[:], r(nb_row[:, k * 128:(k + 1) * 128]),
                             r(ones_row[:]), start=False, stop=True)
            t = ln_sb.tile([128, HALF], F32, tag="t", name="t", bufs=2)
            nc.vector.tensor_mul(out=t, in0=u_tiles[k][:], in1=g1p)
            nc.vector.tensor_sub(out=dest_aps[k].bitcast(F32R), in0=t, in1=g2p)

    # attn_out outlives the attention scope: allocate first
    pool_ao = ctx.enter_context(tc.tile_pool(name="pool_ao", bufs=1))
    attn_out = [pool_ao.tile([128, T], F32, tag=f"ao{k}", name=f"ao{k}")
                for k in range(KD)]
    attn_outB = [pool_ao.tile([128, T], BF16, tag=f"aob{k}", name=f"aob{k}")
                 for k in range(KD)]

    with tc.tile_pool(name="pool_x", bufs=1) as pool_x, \
         tc.tile_pool(name="pool_ctx", bufs=1) as pool_ctx:
        xT = [pool_x.tile([128, TH], F32, tag=f"xT{k}", name=f"xT{k}")
              for k in range(KD)]
        xTb = [pool_x.tile([128, TH], BF16, tag=f"xTb{k}", name=f"xTb{k}")
               for k in range(KD)]
        xgT = [pool_x.tile([128, NG], BF16, tag=f"xgT{k}", name=f"xgT{k}")
               for k in range(KD)]
        ctx_raw = [pool_ctx.tile([128, T], BF16, tag=f"ctx{k}", name=f"ctx{k}")
                   for k in range(KD)]

        with tc.tile_pool(name="pool_qkv", bufs=1) as pool_qkv:
            # ---------------- load x, transpose to feature-major ------------
            with tc.tile_pool(name="xload", bufs=3) as xload, \
                 tc.tile_pool(name="tp_ps", bufs=3, space="PSUM") as tp_ps:
                for ti in range(TH // 128):
                    xtile = xload.tile([128, D], F32, tag="xtile", name="xtile")
                    nc.sync.dma_start(out=xtile,
                                      in_=ap["x_halo"][ti * 128:(ti + 1) * 128, :])
                    for k in range(KD):
                        ps = tp_ps.tile([128, 128], F32, tag="tp", name="tp")
                        nc.tensor.transpose(ps[:], xtile[:, k * 128:(k + 1) * 128],
                                            ident[:])
                        nc.scalar.activation(out=xT[k][:, ti * 128:(ti + 1) * 128],
                                             in_=ps, func=AF.Copy)
                        nc.vector.tensor_copy(
                            out=xTb[k][:, ti * 128:(ti + 1) * 128], in_=ps)
                xg = xload.tile([NG, D], F32, tag="xg", name="xg")
                nc.sync.dma_start(out=xg, in_=ap["x_glob"])
                for k in range(KD):
                    ps = tp_ps.tile([128, NG], F32, tag="tpg", name="tpg")
                    nc.tensor.transpose(ps[:], xg[:, k * 128:(k + 1) * 128],
                                        ident[0:NG, 0:NG])
                    nc.scalar.activation(out=xgT[k], in_=ps, func=AF.Copy)

            if stop_after == "x":
                dummy_out(pool_qkv, None)
                return
            # ---------------- projections (q/k in bf16, v in bf16) ----------
            qT = [pool_qkv.tile([128, T], BF16, tag=f"qT{k}", name=f"qT{k}")
                  for k in range(KD)]
            kT = [pool_qkv.tile([128, TH], BF16, tag=f"kT{k}", name=f"kT{k}")
                  for k in range(KD)]
            # v: token-major per halo chunk, heads interleaved with a ones col:
            # col h*65+d = v[tok, h, d], col h*65+64 = 1.0
            v_sb = [pool_qkv.tile([128, H * 65], BF16, tag=f"v{j}", name=f"v{j}")
                    for j in range(NJ)]
            vg_sb = pool_qkv.tile([NG, H * 65], BF16, tag="vg", name="vg")
            qgT = [pool_qkv.tile([128, NG], BF16, tag=f"qgT{k}", name=f"qgT{k}")
                   for k in range(KD)]
            kgT = [pool_qkv.tile([128, NG], BF16, tag=f"kgT{k}", name=f"kgT{k}")
                   for k in range(KD)]
            masks = []
            for j in range(NJ):
                m = pool_qkv.tile([128, 3 * W], BF16, tag=f"mask{j}",
                                  name=f"mask{j}")
                nc.sync.dma_start(out=m, in_=ap["mask_all"][j])
                masks.append(m)

            with tc.tile_pool(name="wload", bufs=2) as wload, \
                 tc.tile_pool(name="vtmp_sb", bufs=3) as vtmp_sb, \
                 tc.tile_pool(name="proj_ps", bufs=2, space="PSUM") as proj_ps, \
                 tc.tile_pool(name="vproj_ps", bufs=2, space="PSUM") as vproj_ps:
                for wname, bias_sb, dest, gdest, ncols, coff in (
                        ("Wq", bq_sb, qT, qgT, T, W), ("Wk", bk_sb, kT, kgT, TH, 0)):
                    wt = [wload.tile([128, D], BF16, tag=f"w{k}",
                                     name=f"w{wname}{k}") for k in range(KD)]
                    for k in range(KD):
                        nc.sync.dma_start(out=wt[k],
                                          in_=ap[wname][k * 128:(k + 1) * 128, :])
                    for o in range(KD):
                        for c0 in range(0, ncols, HALF):
                            cw = min(HALF, ncols - c0)
                            ps = proj_ps.tile([128, HALF], F32, tag="proj",
                                              name="proj")
                            for k in range(KD):
                                nc.tensor.matmul(
                                    ps[:, :cw], wt[k][:, o * 128:(o + 1) * 128],
                                    xTb[k][:, coff + c0:coff + c0 + cw],
                                    start=(k == 0), stop=(k == KD - 1))
                            nc.scalar.activation(out=dest[o][:, c0:c0 + cw],
                                                 in_=ps[:, :cw], func=AF.Identity,
                                                 bias=bias_sb[:, o:o + 1])
                        psg = proj_ps.tile([128, NG], F32, tag="projg", name="projg")
                        for k in range(KD):
                            nc.tensor.matmul(psg[:],
                                             wt[k][:, o * 128:(o + 1) * 128],
                                             xgT[k], start=(k == 0),
                                             stop=(k == KD - 1))
                        nc.scalar.activation(out=gdest[o], in_=psg, func=AF.Identity,
                                             bias=bias_sb[:, o:o + 1])
                wv = [wload.tile([128, D], BF16, tag=f"w{k}", name=f"wWv{k}")
                      for k in range(KD)]
                for k in range(KD):
                    nc.sync.dma_start(out=wv[k],
                                      in_=ap["Wv"][k * 128:(k + 1) * 128, :])

                def v_project(src_tiles, n_tok, dest):
                    ps = vproj_ps.tile([128, D], F32, tag="vproj", name="vproj")
                    for c0 in range(0, D, HALF):
                        cw = min(HALF, D - c0)
                        for k in range(KD):
                            nc.tensor.matmul(ps[:n_tok, c0:c0 + cw], src_tiles[k],
                                             wv[k][:, c0:c0 + cw],
                                             start=(k == 0), stop=(k == KD - 1))
                    tmp = vtmp_sb.tile([128, D], F32, tag="vtmp", name="vtmp")
                    nc.vector.tensor_add(out=tmp[:n_tok], in0=ps[:n_tok],
                                         in1=bv_bc[:n_tok])
                    dv = dest[:n_tok].rearrange("p (h e) -> p h e", e=65)
                    nc.vector.tensor_copy(
                        out=dv[:, :, 0:64],
                        in_=tmp[:n_tok].rearrange("p (h d) -> p h d", d=DH))
                    nc.vector.memset(dv[:, :, 64:65], 1.0)

                for j in range(NJ):
                    v_project([xTb[k][:, j * 128:(j + 1) * 128] for k in range(KD)],
                              128, v_sb[j])
                v_project(xgT, NG, vg_sb)

            if stop_after == "qkv":
                dummy_out(pool_qkv, None)
                return
            # ---------------- attention ----------------
            def h_slice(t_list, h, cols):
                return t_list[h // 2][(h % 2) * DH:(h % 2) * DH + DH, cols]

            with tc.tile_pool(name="esb", bufs=3) as esb, \
                 tc.tile_pool(name="egsb", bufs=2) as egsb, \
                 tc.tile_pool(name="epsb", bufs=2) as epsb, \
                 tc.tile_pool(name="ogsb", bufs=2) as ogsb, \
                 tc.tile_pool(name="rcsb", bufs=3) as rcsb, \
                 tc.tile_pool(name="sc_ps", bufs=2, space="PSUM") as sc_ps, \
                 tc.tile_pool(name="av_ps", bufs=2, space="PSUM") as av_ps, \
                 tc.tile_pool(name="gsc_ps", bufs=1, space="PSUM") as gsc_ps, \
                 tc.tile_pool(name="div_ps", bufs=2, space="PSUM") as div_ps, \
                 tc.tile_pool(name="pg_ps", bufs=1, space="PSUM") as pg_ps:
                for h in range(H):
                    # global-key scores for all owned queries
                    eg = egsb.tile([NG, T], BF16, tag="eg", name="eg")
                    for c0 in range(0, T, HALF):
                        gps = gsc_ps.tile([NG, HALF], F32, tag="gsc", name="gsc")
                        nc.tensor.matmul(gps[:], h_slice(kgT, h, slice(0, NG)),
                                         h_slice(qT, h, slice(c0, c0 + HALF)),
                                         start=True, stop=True)
                        nc.scalar.activation(out=eg[:, c0:c0 + HALF], in_=gps,
                                             func=AF.Exp, bias=amg_sb[:],
                                             scale=ISCALE)

                    # band scores, transposed: k-chunk j vs queries j-1..j+1
                    e_tiles = {}
                    for j in range(-1, NCH + 1):
                        jdx = j + 1
                        cs = [c for c in (j - 1, j, j + 1) if 0 <= c < NCH]
                        wj = 128 * len(cs)
                        q_lo = cs[0] * 128
                        ps = sc_ps.tile([128, 3 * W], F32, tag="sc", name="sc")
                        nc.tensor.matmul(
                            ps[:, :wj],
                            h_slice(kT, h, slice(jdx * 128, jdx * 128 + 128)),
                            h_slice(qT, h, slice(q_lo, q_lo + wj)),
                            start=True, stop=True)
                        et = esb.tile([128, 3 * W], BF16, tag=f"e{jdx % 4}",
                                      name=f"e{jdx % 4}")
                        nc.scalar.activation(out=et[:, :wj], in_=ps[:, :wj],
                                             func=AF.Exp,
                                             bias=am_sb[:, jdx:jdx + 1],
                                             scale=ISCALE)
                        nc.vector.tensor_mul(out=et[:, :wj], in0=et[:, :wj],
                                             in1=masks[jdx][:, :wj])
                        e_tiles[j] = (et, cs)

                    for c in range(NCH):
                        pav = av_ps.tile([65, 128], F32, tag="av", name="av")
                        for i, j in enumerate((c - 1, c, c + 1)):
                            et, cs = e_tiles[j]
                            off = cs.index(c) * 128
                            nc.tensor.matmul(pav[:],
                                             v_sb[j + 1][:, h * 65:h * 65 + 65],
                                             et[:, off:off + 128],
                                             start=(i == 0), stop=False)
                        nc.tensor.matmul(pav[:], vg_sb[:, h * 65:h * 65 + 65],
                                         eg[:, c * 128:(c + 1) * 128],
                                         start=False, stop=True)
                        # divide by sum-exp row: reciprocal -> broadcast -> mul
                        rcp = rcsb.tile([1, 128], F32, tag="rcp", name="rcp")
                        with nc.allow_low_precision(reason="f32r rounding only"):
                            nc.vector.reciprocal(out=rcp[:].bitcast(F32R),
                                                 in_=pav[64:65])
                        bc = div_ps.tile([DH, 128], F32, tag="bc", name="bc")
                        nc.tensor.matmul(bc[:], r(ones_row64[:]), r(rcp[:]),
                                         start=True, stop=True)
                        csl = h_slice(ctx_raw, h, slice(c * 128, (c + 1) * 128))
                        nc.vector.tensor_copy(out=csl, in_=pav[0:DH])
                        nc.vector.tensor_mul(out=csl, in0=csl, in1=bc)

                    # global-query partials over owned keys
                    pg = pg_ps.tile([65, NG], F32, tag="pg", name="pg")
                    for j in range(NCH):
                        jdx = j + 1
                        ps = sc_ps.tile([128, 3 * W], F32, tag="sc", name="sc")
                        nc.tensor.matmul(
                            ps[:, :NG],
                            h_slice(kT, h, slice(jdx * 128, jdx * 128 + 128)),
                            h_slice(qgT, h, slice(0, NG)),
                            start=True, stop=True)
                        ep = epsb.tile([128, NG], BF16, tag="ep", name="ep")
                        nc.scalar.activation(out=ep, in_=ps[:, :NG], func=AF.Exp,
                                             bias=am_sb[:, jdx:jdx + 1],
                                             scale=ISCALE)
                        nc.tensor.matmul(pg[:], v_sb[jdx][:, h * 65:h * 65 + 65],
                                         ep[:], start=(j == 0),
                                         stop=(j == NCH - 1))
                    ogt = ogsb.tile([65, NG], F32, tag="og", name="og")
                    nc.vector.tensor_copy(out=ogt, in_=pg)
                    nc.sync.dma_start(out=ap["og"][h], in_=ogt)

        if stop_after == "attn":
            dummy_out(pool_ctx, None, og_too=False)
            return
        # ---------------- Wo projection + residual + LN1 ----------------
        with tc.tile_pool(name="wo_load", bufs=1) as wo_load, \
             tc.tile_pool(name="u_sb", bufs=1) as u_sb, \
             tc.tile_pool(name="ln_sb", bufs=1) as ln_sb, \
             tc.tile_pool(name="ln_rows", bufs=1) as ln_rows, \
             tc.tile_pool(name="wo_ps", bufs=2, space="PSUM") as wo_ps, \
             tc.tile_pool(name="row_ps", bufs=1, space="PSUM") as row_ps, \
             tc.tile_pool(name="ln_ps", bufs=2, space="PSUM") as ln_ps:
            g1_row, nb1_row = load_ln_rows(ln_rows, "ln1_g", "ln1_b")
            wo = [wo_load.tile([128, D], BF16, tag=f"wo{k}", name=f"wo{k}")
                  for k in range(KD)]
            for k in range(KD):
                nc.sync.dma_start(out=wo[k], in_=ap["Wo"][k * 128:(k + 1) * 128, :])
            for c0 in range(0, T, HALF):
                u_tiles = []
                for o in range(KD):
                    ps = wo_ps.tile([128, HALF], F32, tag="wops", name="wops")
                    for k in range(KD):
                        nc.tensor.matmul(ps[:], wo[k][:, o * 128:(o + 1) * 128],
                                         ctx_raw[k][:, c0:c0 + HALF],
                                         start=(k == 0), stop=(k == KD - 1))
                    u = u_sb.tile([128, HALF], F32, tag=f"u{o}", name=f"u{o}")
                    nc.scalar.activation(out=u[:].bitcast(F32R), in_=ps,
                                         func=AF.Identity,
                                         bias=bo_sb[:, o:o + 1])
                    nc.vector.tensor_add(out=u[:].bitcast(F32R), in0=u,
                                         in1=xT[o][:, W + c0:W + c0 + HALF])
                    u_tiles.append(u)
                layernorm(u_tiles, g1_row, nb1_row,
                          [attn_out[k][:, c0:c0 + HALF] for k in range(KD)],
                          ln_ps, ln_sb, row_ps)
                for k in range(KD):
                    nc.vector.tensor_copy(out=attn_outB[k][:, c0:c0 + HALF],
                                          in_=attn_out[k][:, c0:c0 + HALF])

    if stop_after == "wo":
        dummy_out(pool_ao, None, og_too=False)
        return
    # ---------------- FFN (Wi resident, Wo2 streamed) ----------------
    with tc.tile_pool(name="u2_sb", bufs=1) as u2_sb:
        u2_all = {}
        with tc.tile_pool(name="wi_load", bufs=1) as wi_load, \
             tc.tile_pool(name="wo2_load", bufs=3) as wo2_load, \
             tc.tile_pool(name="inter_sb", bufs=3) as inter_sb, \
             tc.tile_pool(name="ffn_ps", bufs=2, space="PSUM") as ffn_ps, \
             tc.tile_pool(name="o2_ps", bufs=1, space="PSUM") as o2_ps:
            wi = [wi_load.tile([128, FF], BF16, tag=f"wi{k}", name=f"wi{k}")
                  for k in range(KD)]
            for k in range(KD):
                nc.sync.dma_start(out=wi[k], in_=ap["Wi"][k * 128:(k + 1) * 128, :])
            for c0 in range(0, T, HALF):
                o2 = o2_ps.tile([128, KD, HALF], F32, tag="o2", name="o2")
                for f in range(KF):
                    wo2 = wo2_load.tile([128, D], BF16, tag=f"wo2_{f % 3}",
                                        name=f"wo2_{f % 3}")
                    nc.sync.dma_start(out=wo2,
                                      in_=ap["Wo2"][f * 128:(f + 1) * 128, :])
                    ps = ffn_ps.tile([128, HALF], F32, tag="ffn", name="ffn")
                    for k in range(KD):
                        nc.tensor.matmul(ps[:], wi[k][:, f * 128:(f + 1) * 128],
                                         attn_outB[k][:, c0:c0 + HALF],
                                         start=(k == 0), stop=(k == KD - 1))
                    it = inter_sb.tile([128, HALF], BF16, tag="it", name="it")
                    nc.scalar.activation(out=it, in_=ps, func=gelu_f,
                                         bias=bi_sb[:, f:f + 1])
                    for o in range(KD):
                        nc.tensor.matmul(o2[:, o, :],
                                         wo2[:, o * 128:(o + 1) * 128], it[:],
                                         start=(f == 0), stop=(f == KF - 1))
                for o in range(KD):
                    u = u2_sb.tile([128, HALF], F32, tag=f"u2_{c0}_{o}",
                                   name=f"u2_{c0}_{o}")
                    nc.scalar.activation(out=u[:].bitcast(F32R), in_=o2[:, o, :],
                                         func=AF.Identity,
                                         bias=bo2_sb[:, o:o + 1])
                    nc.vector.tensor_add(out=u[:].bitcast(F32R), in0=u,
                                         in1=attn_out[o][:, c0:c0 + HALF])
                    u2_all[(c0, o)] = u

        # ---------------- LN2 -> output DMA ----------------
        with tc.tile_pool(name="ln_sb2", bufs=1) as ln_sb2, \
             tc.tile_pool(name="ln_rows2", bufs=1) as ln_rows2, \
             tc.tile_pool(name="out_sb", bufs=2) as out_sb, \
             tc.tile_pool(name="row_ps2", bufs=1, space="PSUM") as row_ps2, \
             tc.tile_pool(name="ln_ps2", bufs=2, space="PSUM") as ln_ps2:
            g2_row, nb2_row = load_ln_rows(ln_rows2, "ln2_g", "ln2_b")
            for c0 in range(0, T, HALF):
                dest = [out_sb.tile([128, HALF], F32, tag=f"ot{k}", name=f"ot{k}")
                        for k in range(KD)]
                layernorm([u2_all[(c0, o)] for o in range(KD)], g2_row, nb2_row,
                          [d[:] for d in dest], ln_ps2, ln_sb2, row_ps2)
                for k in range(KD):
                    nc.sync.dma_start(out=ap["outT"][k * 128:(k + 1) * 128,
                                                     c0:c0 + HALF], in_=dest[k])


def build_nc(sim_mode=False, repeat=1, stop_after=None):
    from contextlib import ExitStack
    nc = bacc.Bacc("TRN2", target_bir_lowering=False, debug=False)
    ap = {}
    ap["x_halo"] = nc.dram_tensor("x_halo", [TH, D], F32, kind="ExternalInput").ap()
    ap["x_glob"] = nc.dram_tensor("x_glob", [NG, D], F32, kind="ExternalInput").ap()
    ap["am_halo"] = nc.dram_tensor("am_halo", [TH], F32, kind="ExternalInput").ap()
    ap["am_glob"] = nc.dram_tensor("am_glob", [NG], F32, kind="ExternalInput").ap()
    ap["mask_all"] = nc.dram_tensor("mask_all", [NJ, 128, 3 * W], BF16,
                                    kind="ExternalInput").ap()
    for n, sh in (("Wq", [D, D]), ("Wk", [D, D]), ("Wv", [D, D]), ("Wo", [D, D]),
                  ("Wi", [D, FF]), ("Wo2", [FF, D])):
        ap[n] = nc.dram_tensor(n, sh, BF16, kind="ExternalInput").ap()
    for n, sh in (("bq", [D]), ("bk", [D]), ("bv", [D]), ("bo", [D]),
                  ("bi", [FF]), ("bo2", [D]), ("ln1_b", [D]),
                  ("ln2_b", [D]),):
        ap[n] = nc.dram_tensor(n, sh, F32, kind="ExternalInput").ap()
    for n in ("ln1_g", "ln2_g"):
        ap[n] = nc.dram_tensor(n, [D], F32R, kind="ExternalInput").ap()
    ap["outT"] = nc.dram_tensor("outT", [D, T], F32, kind="ExternalOutput").ap()
    ap["og"] = nc.dram_tensor("og", [H, 65, NG], F32, kind="ExternalOutput").ap()

    with tile.TileContext(nc) as tc:
        if repeat > 1:
            def body(i):
                with ExitStack() as c2:
                    build_body(nc, tc, ap, c2, sim_mode, stop_after)
            tc.For_i_unrolled(0, repeat, 1, body, max_unroll=1)
        else:
            with ExitStack() as c2:
                build_body(nc, tc, ap, c2, sim_mode, stop_after)
    nc.compile()
    return nc


# ---------------- host side ----------------

def shard_inputs(inputs):
    hs = np.asarray(inputs["hidden_states"], np.float32)
    am = np.asarray(inputs["attention_mask"], np.float32)
    shared = {}
    for n in ("Wq", "bq", "Wk", "bk", "Wv", "bv", "Wo", "bo", "ln1_g", "ln1_b",
              "Wi", "bi", "Wo2", "bo2", "ln2_g", "ln2_b"):
        shared[n] = np.ascontiguousarray(np.asarray(inputs[n], np.float32))
    for n in ("Wq", "Wk", "Wv", "Wo", "Wi", "Wo2"):
        shared[n] = shared[n].astype(ml_dtypes.bfloat16)
    in_maps = []
    for core in range(N_CORES):
        b, q0 = core // 4, (core % 4) * T
        xh = np.zeros((TH, D), np.float32)
        amh = np.zeros((TH,), np.float32)
        lo, hi = q0 - W, q0 + T + W
        slo, shi = max(lo, 0), min(hi, S)
        xh[slo - lo:shi - lo] = hs[b, slo:shi]
        amh[slo - lo:shi - lo] = am[b, slo:shi]
        mask = np.zeros((NJ, 128, 3 * W), np.float32)
        for j in range(-1, NCH + 1):
            cs = [c for c in (j - 1, j, j + 1) if 0 <= c < NCH]
            kpos = q0 + j * 128 + np.arange(128)[:, None]
            for i, c in enumerate(cs):
                qpos = q0 + c * 128 + np.arange(128)[None, :]
                valid = (np.abs(kpos - qpos) <= W) & (kpos >= NG) & (kpos >= 0) \
                    & (kpos < S)
                mask[j + 1, :, i * 128:(i + 1) * 128] = valid
        m = {"x_halo": xh, "x_glob": np.ascontiguousarray(hs[b, :NG]),
             "am_halo": amh, "am_glob": np.ascontiguousarray(am[b, :NG]),
             "mask_all": mask.astype(ml_dtypes.bfloat16)}
        m.update(shared)
        in_maps.append(m)
    return in_maps


def _np_layernorm(x, g, b):
    mu = x.mean(-1, keepdims=True)
    var = ((x - mu) ** 2).mean(-1, keepdims=True)
    return (x - mu) / np.sqrt(var + EPS) * g + b


def _np_gelu(x):
    from scipy.special import erf
    return x * 0.5 * (1.0 + erf(x / np.sqrt(2.0)))


def host_tail(inputs, og_by_core, sim_mode=False):
    """Combine global-query flash partials; dense tail for the global rows."""
    hs = np.asarray(inputs["hidden_states"], np.float64)
    rows = np.zeros((B, NG, D))
    for b in range(B):
        o = sum(np.asarray(og_by_core[4 * b + c], np.float64) for c in range(4))
        gctx = o[:, :DH, :] / o[:, 64:65, :]          # [H, DH, NG]
        gctx = gctx.transpose(2, 0, 1).reshape(NG, D)  # feature index = h*64+d
        u = gctx @ np.asarray(inputs["Wo"], np.float64) \
            + np.asarray(inputs["bo"], np.float64) + hs[b, :NG]
        a = _np_layernorm(u, np.asarray(inputs["ln1_g"], np.float64),
                          np.asarray(inputs["ln1_b"], np.float64))
        inter = a @ np.asarray(inputs["Wi"], np.float64) \
            + np.asarray(inputs["bi"], np.float64)
        if not sim_mode:
            inter = _np_gelu(inter)
        u2 = inter @ np.asarray(inputs["Wo2"], np.float64) \
            + np.asarray(inputs["bo2"], np.float64) + a
        rows[b] = _np_layernorm(u2, np.asarray(inputs["ln2_g"], np.float64),
                                np.asarray(inputs["ln2_b"], np.float64))
    return rows.astype(np.float32)


def assemble(inputs, results, sim_mode=False):
    out = np.zeros((B, S, D), np.float32)
    for core in range(N_CORES):
        b, q0 = core // 4, (core % 4) * T
        out[b, q0:q0 + T] = np.asarray(results[core]["outT"]).T
    out[:, :NG] = host_tail(inputs, [results[c]["og"] for c in range(N_CORES)],
                            sim_mode)
    return out


def kernel(**inputs):
    from concourse import bass_utils
    if "nc" not in _nc_cache:
        _nc_cache["nc"] = build_nc()
    nc = _nc_cache["nc"]
    in_maps = shard_inputs(inputs)
    res = bass_utils.run_bass_kernel_spmd(nc, in_maps, core_ids=list(range(N_CORES)))
    return assemble(inputs, res.results)



# revision 3
# speedup vs baseline: 1.2064x; 1.2064x over previous
"""Longformer layer v2 (B=2, S=4096, D=768, H=12, w=128, NG=32) on 8 TRN2 cores.

Sharding: sequence-parallel. Core c owns tokens [q0, q0+1024) of batch b=c//4.
Each core computes band+global-key attention and the dense pipeline for its
1024 tokens; global-query rows emit flash partials combined on host.

v2 changes vs baseline:
- x arrives pre-transposed (feature-major) and pre-bf16 from the host: the
  on-device PE transpose phase is gone.
- All weights (Wq..Wo2) are DMA'd into resident SBUF tiles at kernel start so
  loads overlap early compute; Wo2 is no longer streamed twice.
- Attention processes heads in PAIRS. Head h=2p lives on partitions 0:64 of
  feature tile p, h=2p+1 on 64:128, so paired score matmuls land on disjoint
  PE row groups and run concurrently. Scores for both heads of a pair go to
  one 2-bank psum tile -> ONE exp ACT per key-chunk covers both heads.
- Global-query (pg) scores are extra columns of the band score psum (no
  separate exp); pg AV matmuls reuse the band AV stationary.
- Band AV accumulates per 512-query group [65,512] psum: the global-key AV
  (full span, start=True) runs first, the 6 clipped band contributions then
  pure-accumulate. 14 matmuls/head instead of 32.
- Softmax normalization per (head, group): one reciprocal [1,512]->bf16, a
  DMA partition-broadcast (gpsimd) to SBUF, copy+mul on DVE.
- LayerNorm: sums via col-tiled paired matmuls (s1/s2 in one bank), rstd/mr
  broadcast via gpsimd DMA, gain/bias applied with a fused tensor_scalar.
  Residuals ride in bf16; the residual adds are folded into the Wo/Wo2
  accumulation as identity matmuls.
"""

import math
import numpy as np
import ml_dtypes

import concourse.bacc as bacc
import concourse.mybir as mybir
import concourse.tile as tile
from concourse.masks import make_identity

F32 = mybir.dt.float32
BF16 = mybir.dt.bfloat16
AF = mybir.ActivationFunctionType
ALU = mybir.AluOpType

B, S, D, FF = 2, 4096, 768, 3072
H, DH, W, NG = 12, 64, 128, 32
EPS = 1e-12
T = 1024
TH = T + 2 * W
KD = D // 128       # 6
KF = FF // 128      # 24
NCH = T // W        # 8 owned chunks
NJ = NCH + 2        # key chunks jdx = 0..9 (j = jdx-1)
HALF = 512
N_CORES = 8
ISCALE = 1.0 / math.sqrt(DH)

# band window per key chunk j: query chunks [j-1, j, j+1] clipped to [0, 8)
WIN = {}
for j in range(-1, NCH + 1):
    cs = [c for c in (j - 1, j, j + 1) if 0 <= c < NCH]
    WIN[j] = (cs[0] * 128, len(cs) * 128)   # (q_lo, wj)

_nc_cache = {}


def build_body(nc, tc, ap, ctx, sim_mode=False):
    import os
    stop_after = os.environ.get("K2_STOP", "")
    gelu_f = AF.Identity if sim_mode else AF.Gelu
    persist = ctx.enter_context(tc.tile_pool(name="persist", bufs=1))

    def dummy_out(pool, og_too=True):
        z = pool.tile([128, T], F32, tag="zdum", name="zdum")
        nc.vector.memset(z, 0.0)
        for k in range(KD):
            nc.sync.dma_start(out=ap["outT"][k * 128:(k + 1) * 128, :], in_=z)
        if og_too:
            zg = pool.tile([65, NG], F32, tag="zgdum", name="zgdum")
            nc.vector.memset(zg, 1.0)
            for h in range(H):
                nc.sync.dma_start(out=ap["og"][h], in_=zg)

    # Resident weights for the late phases (Wo/Wi/Wo2). Tiles allocated here;
    # their DMAs are issued after the first-needed loads (x, Wq/Wk/Wv) so the
    # DMA queues serve the projection phase first.
    wo = [persist.tile([128, D], BF16, tag=f"wo{k}", name=f"wo{k}")
          for k in range(KD)]
    wi = [persist.tile([128, FF], BF16, tag=f"wi{k}", name=f"wi{k}")
          for k in range(KD)]
    wo2 = [persist.tile([128, D], BF16, tag=f"wo2_{f}", name=f"wo2_{f}")
           for f in range(KF)]

    def load_late_weights():
        for k in range(KD):
            nc.sync.dma_start(out=wo[k], in_=ap["Wo"][k * 128:(k + 1) * 128, :])
        for k in range(KD):
            nc.sync.dma_start(out=wi[k], in_=ap["Wi"][k * 128:(k + 1) * 128, :])
        for f in range(KF):
            nc.sync.dma_start(out=wo2[f], in_=ap["Wo2"][f * 128:(f + 1) * 128, :])

    # ---------------- constants / biases ----------------
    identB = persist.tile([128, 128], BF16, tag="identB", name="identB")
    make_identity(nc, identB)
    ones_col = persist.tile([128, 1], BF16, tag="ones_col", name="ones_col")
    nc.vector.memset(ones_col, 1.0)
    ones_row = persist.tile([1, 128], BF16, tag="ones_row", name="ones_row")
    nc.vector.memset(ones_row, 1.0)
    eps_sc = persist.tile([1, 1], F32, tag="eps_sc", name="eps_sc")
    nc.vector.memset(eps_sc, EPS)

    def load_cols(name, n):
        t = persist.tile([128, n], F32, tag=name)
        nc.sync.dma_start(out=t, in_=ap[name].rearrange("(k p) -> p k", p=128))
        return t

    bq_sb = load_cols("bq", KD)
    bk_sb = load_cols("bk", KD)
    bo_sb = load_cols("bo", KD)
    bi_sb = load_cols("bi", KF)
    bo2_sb = load_cols("bo2", KD)
    g1_sb = load_cols("ln1_g", KD)
    b1_sb = load_cols("ln1_b", KD)
    g2_sb = load_cols("ln2_g", KD)
    b2_sb = load_cols("ln2_b", KD)

    bv_bc = persist.tile([128, D], BF16, tag="bv_bc", name="bv_bc")
    nc.gpsimd.dma_start(out=bv_bc, in_=ap["bv"].unsqueeze(0).partition_broadcast(128))

    am_sb = persist.tile([128, NJ], F32, tag="am_sb", name="am_sb")
    nc.sync.dma_start(out=am_sb, in_=ap["am_halo"].rearrange("(k p) -> p k", p=128))
    amg_sb = persist.tile([64, 1], F32, tag="amg_sb", name="amg_sb")
    nc.sync.dma_start(out=amg_sb, in_=ap["am_glob2"].unsqueeze(1))

    # ---------------- LayerNorm (feature-major, bf16 residual stream) ------
    def layernorm(u_tiles, cols, g_sb, b_sb, dest_aps, pools):
        """u_tiles: KD bf16 [128, cols] SBUF tiles. dest_aps[k]: [128, cols]."""
        usq_sb, row_sb, s_ps, bc_ps = pools
        s = s_ps.tile([33, HALF], F32, tag="s", name="s")
        for k in range(KD):
            usq = usq_sb.tile([128, HALF], BF16, tag=f"usq{k % 2}",
                              name=f"usq{k % 2}", bufs=2)
            nc.vector.tensor_mul(out=usq[:, :cols], in0=u_tiles[k][:, :cols],
                                 in1=u_tiles[k][:, :cols])
            nc.tensor.matmul(s[0:1, :cols], ones_col, u_tiles[k][:, :cols],
                             start=(k == 0), stop=(k == KD - 1),
                             tile_position=(0, 0))
            nc.tensor.matmul(s[32:33, :cols], ones_col, usq[:, :cols],
                             start=(k == 0), stop=(k == KD - 1),
                             tile_position=(0, 32))
        mu = row_sb.tile([1, HALF], F32, tag="mu", name="mu")
        nc.vector.tensor_scalar_mul(out=mu[:, :cols], in0=s[0:1, :cols],
                                    scalar1=1.0 / D)
        q = row_sb.tile([1, HALF], F32, tag="q", name="q")
        nc.vector.tensor_mul(out=q[:, :cols], in0=s[0:1, :cols], in1=mu[:, :cols])
        vD = row_sb.tile([1, HALF], F32, tag="vD", name="vD")
        nc.vector.tensor_sub(out=vD[:, :cols], in0=s[32:33, :cols], in1=q[:, :cols])
        sd = row_sb.tile([1, HALF], F32, tag="sd", name="sd")
        nc.scalar.activation(out=sd[:, :cols], in_=vD[:, :cols], func=AF.Sqrt,
                             bias=eps_sc[:], scale=1.0 / D)
        rstd = row_sb.tile([1, HALF], BF16, tag="rstd", name="rstd")
        with nc.allow_low_precision(reason="bf16 norm scales"):
            nc.vector.reciprocal(out=rstd[:, :cols], in_=sd[:, :cols])
            mr = row_sb.tile([1, HALF], BF16, tag="mr", name="mr")
            nc.vector.tensor_mul(out=mr[:, :cols], in0=mu[:, :cols],
                                 in1=rstd[:, :cols])
        rstd_bc = bc_ps.tile([128, HALF], F32, tag="rstd_bc", name="rstd_bc")
        nc.tensor.matmul(rstd_bc[:, :cols], ones_row, rstd[:, :cols],
                         start=True, stop=True)
        mr_bc = bc_ps.tile([128, HALF], F32, tag="mr_bc", name="mr_bc")
        nc.tensor.matmul(mr_bc[:, :cols], ones_row, mr[:, :cols],
                         start=True, stop=True)
        for k in range(KD):
            w = usq_sb.tile([128, HALF], BF16, tag=f"w{k % 2}",
                            name=f"w{k % 2}", bufs=2)
            nc.vector.tensor_mul(out=w[:, :cols], in0=u_tiles[k][:, :cols],
                                 in1=rstd_bc[:, :cols])
            nc.vector.tensor_sub(out=w[:, :cols], in0=w[:, :cols],
                                 in1=mr_bc[:, :cols])
            with nc.allow_low_precision(reason="bf16 ln out"):
                nc.vector.tensor_scalar(
                    out=dest_aps[k], in0=w[:, :cols],
                    scalar1=g_sb[:, k:k + 1], scalar2=b_sb[:, k:k + 1],
                    op0=ALU.mult, op1=ALU.add)

    # attn_outB / u2 outlive inner scopes
    pool_ao = ctx.enter_context(tc.tile_pool(name="pool_ao", bufs=1))
    attn_outB = [pool_ao.tile([128, T], BF16, tag=f"aob{k}", name=f"aob{k}")
                 for k in range(KD)]

    with tc.tile_pool(name="pool_x", bufs=1) as pool_x, \
         tc.tile_pool(name="pool_ctx", bufs=1) as pool_ctx:
        xT = [pool_x.tile([128, TH], BF16, tag=f"xT{k}", name=f"xT{k}")
              for k in range(KD)]
        xgT = [pool_x.tile([128, NG], BF16, tag=f"xgT{k}", name=f"xgT{k}")
               for k in range(KD)]
        for k in range(KD):
            nc.sync.dma_start(out=xT[k], in_=ap["xT"][k * 128:(k + 1) * 128, :])
            nc.sync.dma_start(out=xgT[k], in_=ap["xgT"][k * 128:(k + 1) * 128, :])
        ctx_raw = [pool_ctx.tile([128, T], BF16, tag=f"ctx{k}", name=f"ctx{k}")
                   for k in range(KD)]

        with tc.tile_pool(name="pool_qkv", bufs=1) as pool_qkv:
            qT = [pool_qkv.tile([128, T], BF16, tag=f"qT{k}", name=f"qT{k}")
                  for k in range(KD)]
            kT = [pool_qkv.tile([128, TH], BF16, tag=f"kT{k}", name=f"kT{k}")
                  for k in range(KD)]
            qgT = [pool_qkv.tile([128, NG], BF16, tag=f"qgT{k}", name=f"qgT{k}")
                   for k in range(KD)]
            kgT = [pool_qkv.tile([128, NG], BF16, tag=f"kgT{k}", name=f"kgT{k}")
                   for k in range(KD)]
            # v: token-major per halo chunk, heads interleaved with ones col
            v_sb = [pool_qkv.tile([128, H * 65], BF16, tag=f"v{j}", name=f"v{j}")
                    for j in range(NJ)]
            vg2 = pool_qkv.tile([64, H * 65], BF16, tag="vg2", name="vg2")

            with tc.tile_pool(name="wqkv", bufs=1) as wqkv_pool, \
                 tc.tile_pool(name="vtmp_sb", bufs=3) as vtmp_sb, \
                 tc.tile_pool(name="proj_ps", bufs=2, space="PSUM") as proj_ps, \
                 tc.tile_pool(name="vproj_ps", bufs=2, space="PSUM") as vproj_ps:
                wq = [wqkv_pool.tile([128, D], BF16, tag=f"wq{k}",
                                     name=f"wq{k}") for k in range(KD)]
                wk = [wqkv_pool.tile([128, D], BF16, tag=f"wk{k}",
                                     name=f"wk{k}") for k in range(KD)]
                wv = [wqkv_pool.tile([128, D], BF16, tag=f"wv{k}",
                                     name=f"wv{k}") for k in range(KD)]
                for k in range(KD):
                    nc.sync.dma_start(out=wq[k],
                                      in_=ap["Wq"][k * 128:(k + 1) * 128, :])
                    nc.sync.dma_start(out=wk[k],
                                      in_=ap["Wk"][k * 128:(k + 1) * 128, :])
                    nc.sync.dma_start(out=wv[k],
                                      in_=ap["Wv"][k * 128:(k + 1) * 128, :])
                load_late_weights()
                for wt, bias_sb, dest, gdest, ncols, coff in (
                        (wq, bq_sb, qT, qgT, T, W), (wk, bk_sb, kT, kgT, TH, 0)):
                    for o in range(KD):
                        for c0 in range(0, ncols, HALF):
                            cw = min(HALF, ncols - c0)
                            ps = proj_ps.tile([128, HALF], F32, tag="proj",
                                              name="proj")
                            for k in range(KD):
                                nc.tensor.matmul(
                                    ps[:, :cw], wt[k][:, o * 128:(o + 1) * 128],
                                    xT[k][:, coff + c0:coff + c0 + cw],
                                    start=(k == 0), stop=(k == KD - 1))
                            nc.scalar.activation(out=dest[o][:, c0:c0 + cw],
                                                 in_=ps[:, :cw], func=AF.Identity,
                                                 bias=bias_sb[:, o:o + 1])
                        psg = proj_ps.tile([128, NG], F32, tag="projg",
                                           name="projg")
                        for k in range(KD):
                            nc.tensor.matmul(psg[:],
                                             wt[k][:, o * 128:(o + 1) * 128],
                                             xgT[k], start=(k == 0),
                                             stop=(k == KD - 1))
                        nc.scalar.activation(out=gdest[o], in_=psg,
                                             func=AF.Identity,
                                             bias=bias_sb[:, o:o + 1])

                def v_project(src_tiles, n_tok, dest):
                    ps = vproj_ps.tile([128, D], F32, tag="vproj", name="vproj")
                    for c0 in range(0, D, HALF):
                        cw = min(HALF, D - c0)
                        for k in range(KD):
                            nc.tensor.matmul(ps[:n_tok, c0:c0 + cw],
                                             src_tiles[k],
                                             wv[k][:, c0:c0 + cw],
                                             start=(k == 0), stop=(k == KD - 1))
                    tmp = vtmp_sb.tile([128, D], F32, tag="vtmp", name="vtmp")
                    nc.vector.tensor_add(out=tmp[:n_tok], in0=ps[:n_tok],
                                         in1=bv_bc[:n_tok])
                    dv = dest.rearrange("p (h e) -> p h e", e=65)[:n_tok]
                    nc.vector.tensor_copy(
                        out=dv[:, :, 0:64],
                        in_=tmp[:n_tok].rearrange("p (h d) -> p h d", d=DH))
                    nc.vector.memset(dv[:, :, 64:65], 1.0)

                for j in range(NJ):
                    v_project([xT[k][:, j * 128:(j + 1) * 128] for k in range(KD)],
                              128, v_sb[j])
                v_project(xgT, NG, vg2)
                nc.vector.tensor_copy(out=vg2[32:64], in_=vg2[0:32])

            # ---------------- attention ----------------
            if stop_after == "proj":
                dummy_out(pool_qkv)
                return

            def kslice(tiles, h, cols):
                return tiles[h // 2][(h % 2) * DH:(h % 2) * DH + DH, cols]

            with tc.tile_pool(name="mask_sb", bufs=1) as mask_sb_pool, \
                 tc.tile_pool(name="esb", bufs=1) as esb, \
                 tc.tile_pool(name="egsb", bufs=2) as egsb, \
                 tc.tile_pool(name="rcsb", bufs=2) as rcsb, \
                 tc.tile_pool(name="bcsb", bufs=2) as bcsb, \
                 tc.tile_pool(name="ogsb", bufs=2) as ogsb, \
                 tc.tile_pool(name="sc_ps", bufs=2, space="PSUM") as sc_ps, \
                 tc.tile_pool(name="eg_ps", bufs=1, space="PSUM") as eg_ps, \
                 tc.tile_pool(name="av_ps", bufs=2, space="PSUM") as av_ps, \
                 tc.tile_pool(name="pg_ps", bufs=1, space="PSUM") as pg_ps:
                mask_sb = mask_sb_pool.tile([128, NJ, 3 * W], BF16, tag="mask",
                                            name="mask")
                nc.sync.dma_start(out=mask_sb, in_=ap["mask_all"])

                import os as _os
                DIS = set(_os.environ.get("K2_DISABLE", "").split(","))
                for p in range(H // 2):
                    h0, h1 = 2 * p, 2 * p + 1
                    # global-key scores for both heads: [64, T] (h0 rows 0:32)
                    eg = egsb.tile([64, T], BF16, tag=f"eg{p % 2}",
                                   name=f"eg{p % 2}")
                    if "eg" not in DIS:
                        for c0 in range(0, T, HALF):
                            gps = eg_ps.tile([64, HALF], F32, tag="eg", name="eg")
                            # pending-zero tracking is per-partition: each
                            # head's MM owns its own partition range
                            nc.tensor.matmul(gps[0:32, :],
                                             kslice(kgT, h0, slice(0, NG)),
                                             kslice(qT, h0, slice(c0, c0 + HALF)),
                                             start=True, stop=True)
                            nc.tensor.matmul(gps[32:64, :],
                                             kslice(kgT, h1, slice(0, NG)),
                                             kslice(qT, h1, slice(c0, c0 + HALF)),
                                             start=True, stop=True)
                            nc.scalar.activation(out=eg[:, c0:c0 + HALF], in_=gps,
                                                 func=AF.Exp, bias=amg_sb[:],
                                                 scale=ISCALE)
                    else:
                        nc.vector.memset(eg, 0.0)

                    # band + pg scores: one 2-bank psum tile per key chunk
                    e_tiles = {}
                    for j in range(-1, NCH + 1):
                        jdx = j + 1
                        q_lo, wj = WIN[j]
                        has_pg = 1 <= jdx <= NCH
                        wtot = wj + (NG if has_pg else 0)
                        ps = sc_ps.tile([128, 2, HALF], F32, tag="sc", name="sc")
                        for hp, h in ((0, h0), (1, h1)):
                            nc.tensor.matmul(
                                ps[:, hp, :wj],
                                kslice(kT, h, slice(jdx * 128, jdx * 128 + 128)),
                                kslice(qT, h, slice(q_lo, q_lo + wj)),
                                start=True, stop=not has_pg)
                        if has_pg:
                            for hp, h in ((0, h0), (1, h1)):
                                nc.tensor.matmul(
                                    ps[:, hp, wj:wj + NG],
                                    kslice(kT, h, slice(jdx * 128, jdx * 128 + 128)),
                                    kslice(qgT, h, slice(0, NG)),
                                    start=False, stop=True)
                        et = esb.tile([128, 2, 416], BF16, tag=f"e{jdx}",
                                      name=f"e{jdx}")
                        nc.scalar.activation(out=et[:, :, :wtot],
                                             in_=ps[:, :, :wtot], func=AF.Exp,
                                             bias=am_sb[:, jdx:jdx + 1],
                                             scale=ISCALE)
                        for hp in (0, 1):
                            nc.vector.tensor_mul(out=et[:, hp, :wj],
                                                 in0=et[:, hp, :wj],
                                                 in1=mask_sb[:, jdx, :wj])
                        e_tiles[j] = et

                    # AV per head per 512-query group; pg rides along
                    for hp, h in ((0, h0), (1, h1)):
                        pgp = pg_ps.tile([65, NG], F32, tag="pg", name="pg")
                        for g in range(2):
                            g0 = HALF * g
                            pav = av_ps.tile([65, HALF], F32, tag="av", name="av")
                            if "gav" not in DIS:
                                nc.tensor.matmul(
                                    pav[:],
                                    vg2[hp * 32:hp * 32 + 32, h * 65:h * 65 + 65],
                                    eg[hp * 32:hp * 32 + 32, g0:g0 + HALF],
                                    start=True, stop=False)
                            else:
                                nc.tensor.matmul(
                                    pav[:], vg2[0:32, h * 65:h * 65 + 65],
                                    eg[0:32, g0:g0 + HALF],
                                    start=True, stop=False)
                            for j in range(4 * g - 1, 4 * g + 5):
                                jdx = j + 1
                                q_lo, wj = WIN[j]
                                lo = max(q_lo, g0)
                                hi = min(q_lo + wj, g0 + HALF)
                                et = e_tiles[j]
                                nc.tensor.matmul(
                                    pav[:, lo - g0:hi - g0],
                                    v_sb[jdx][:, h * 65:h * 65 + 65],
                                    et[:, hp, lo - q_lo:hi - q_lo],
                                    start=False, stop=(j == 4 * g + 4))
                                if 4 * g <= j <= 4 * g + 3:
                                    nc.tensor.matmul(
                                        pgp[:],
                                        v_sb[jdx][:, h * 65:h * 65 + 65],
                                        et[:, hp, wj:wj + NG],
                                        start=(j == 0), stop=(j == NCH - 1))
                            # normalize: 1/denominator, DRAM round-trip
                            # broadcast (no spare psum banks here), copy+mul
                            rcp = rcsb.tile([1, HALF], BF16, tag="rcp", name="rcp")
                            with nc.allow_low_precision(reason="bf16 softmax div"):
                                nc.vector.reciprocal(out=rcp, in_=pav[64:65, :])
                            ridx = (p * 2 + hp) * 2 + g
                            nc.sync.dma_start(out=ap["scr"][ridx:ridx + 1, :],
                                              in_=rcp[:])
                            # broadcast to all 128 partitions so the mul's two
                            # SBUF inputs share a base partition (walrus rule)
                            bc = bcsb.tile([128, HALF], BF16, tag="bc", name="bc")
                            nc.sync.dma_start(
                                out=bc,
                                in_=ap["scr"][ridx].unsqueeze(0)
                                .partition_broadcast(128))
                            csl = kslice(ctx_raw, h, slice(g0, g0 + HALF))
                            nc.vector.tensor_copy(out=csl, in_=pav[0:64, :])
                            nc.vector.tensor_mul(out=csl, in0=csl,
                                                 in1=bc[hp * 64:hp * 64 + 64, :])
                        ogt = ogsb.tile([65, NG], F32, tag="og", name="og")
                        if "og" not in DIS:
                            nc.vector.tensor_copy(out=ogt, in_=pgp)
                        else:
                            nc.vector.memset(ogt, 1.0)
                        nc.sync.dma_start(out=ap["og"][h], in_=ogt)

        # ---------------- Wo + residual + LN1 ----------------
        if stop_after == "attn":
            dummy_out(pool_ctx, og_too=False)
            return
        with tc.tile_pool(name="u_sb", bufs=1) as u_sb, \
             tc.tile_pool(name="usq_sb", bufs=1) as usq_sb, \
             tc.tile_pool(name="row_sb", bufs=2) as row_sb, \
             tc.tile_pool(name="wo_ps", bufs=2, space="PSUM") as wo_ps, \
             tc.tile_pool(name="s_ps", bufs=2, space="PSUM") as s_ps, \
             tc.tile_pool(name="bc_ps", bufs=1, space="PSUM") as bc_ps:
            ln_pools = (usq_sb, row_sb, s_ps, bc_ps)
            for c0 in range(0, T, HALF):
                u_tiles = []
                for o in range(KD):
                    ps = wo_ps.tile([128, HALF], F32, tag="wops", name="wops")
                    for k in range(KD):
                        nc.tensor.matmul(ps[:], wo[k][:, o * 128:(o + 1) * 128],
                                         ctx_raw[k][:, c0:c0 + HALF],
                                         start=(k == 0), stop=False)
                    nc.tensor.matmul(ps[:], identB,
                                     xT[o][:, W + c0:W + c0 + HALF],
                                     start=False, stop=True)
                    u = u_sb.tile([128, HALF], BF16, tag=f"u{o}", name=f"u{o}")
                    nc.scalar.activation(out=u, in_=ps, func=AF.Identity,
                                         bias=bo_sb[:, o:o + 1])
                    u_tiles.append(u)
                layernorm(u_tiles, HALF, g1_sb, b1_sb,
                          [attn_outB[k][:, c0:c0 + HALF] for k in range(KD)],
                          ln_pools)

    # ---------------- FFN (two passes: inter tiles staged in SBUF) --------
    if stop_after == "wo":
        dummy_out(pool_ao, og_too=False)
        return
    with tc.tile_pool(name="u2_sb", bufs=1) as u2_sb, \
         tc.tile_pool(name="it_sb", bufs=1) as it_sb, \
         tc.tile_pool(name="usq2_sb", bufs=1) as usq2_sb, \
         tc.tile_pool(name="row2_sb", bufs=2) as row2_sb, \
         tc.tile_pool(name="out_sb", bufs=2) as out_sb, \
         tc.tile_pool(name="ffn_ps", bufs=2, space="PSUM") as ffn_ps, \
         tc.tile_pool(name="o2_ps", bufs=2, space="PSUM") as o2_ps, \
         tc.tile_pool(name="s2_ps", bufs=2, space="PSUM") as s2_ps, \
         tc.tile_pool(name="bc2_ps", bufs=1, space="PSUM") as bc2_ps:
        ln2_pools = (usq2_sb, row2_sb, s2_ps, bc2_ps)
        for c0 in range(0, T, HALF):
            its = []
            for f in range(KF):
                ps = ffn_ps.tile([128, HALF], F32, tag="ffn", name="ffn")
                for k in range(KD):
                    nc.tensor.matmul(ps[:], wi[k][:, f * 128:(f + 1) * 128],
                                     attn_outB[k][:, c0:c0 + HALF],
                                     start=(k == 0), stop=(k == KD - 1))
                it = it_sb.tile([128, HALF], BF16, tag=f"it{f}", name=f"it{f}")
                nc.scalar.activation(out=it, in_=ps, func=gelu_f,
                                     bias=bi_sb[:, f:f + 1])
                its.append(it)
            u2_tiles = []
            for o in range(KD):
                ps = o2_ps.tile([128, HALF], F32, tag="o2", name="o2")
                for f in range(KF):
                    nc.tensor.matmul(ps[:], wo2[f][:, o * 128:(o + 1) * 128],
                                     its[f], start=(f == 0), stop=False)
                nc.tensor.matmul(ps[:], identB, attn_outB[o][:, c0:c0 + HALF],
                                 start=False, stop=True)
                u2 = u2_sb.tile([128, HALF], BF16, tag=f"u2_{o}", name=f"u2_{o}")
                nc.scalar.activation(out=u2, in_=ps, func=AF.Identity,
                                     bias=bo2_sb[:, o:o + 1])
                u2_tiles.append(u2)
            dest = [out_sb.tile([128, HALF], F32, tag=f"ot{k}", name=f"ot{k}")
                    for k in range(KD)]
            layernorm(u2_tiles, HALF, g2_sb, b2_sb, [d[:] for d in dest],
                      ln2_pools)
            for k in range(KD):
                nc.sync.dma_start(out=ap["outT"][k * 128:(k + 1) * 128,
                                                 c0:c0 + HALF], in_=dest[k])


def build_nc(sim_mode=False, repeat=1):
    from contextlib import ExitStack
    nc = bacc.Bacc("TRN2", target_bir_lowering=False, debug=False)
    ap = {}
    ap["xT"] = nc.dram_tensor("xT", [D, TH], BF16, kind="ExternalInput").ap()
    ap["xgT"] = nc.dram_tensor("xgT", [D, NG], BF16, kind="ExternalInput").ap()
    ap["am_halo"] = nc.dram_tensor("am_halo", [TH], F32, kind="ExternalInput").ap()
    ap["am_glob2"] = nc.dram_tensor("am_glob2", [64], F32, kind="ExternalInput").ap()
    ap["mask_all"] = nc.dram_tensor("mask_all", [128, NJ, 3 * W], BF16,
                                    kind="ExternalInput").ap()
    for n, sh in (("Wq", [D, D]), ("Wk", [D, D]), ("Wv", [D, D]), ("Wo", [D, D]),
                  ("Wi", [D, FF]), ("Wo2", [FF, D])):
        ap[n] = nc.dram_tensor(n, sh, BF16, kind="ExternalInput").ap()
    for n, sh in (("bq", [D]), ("bk", [D]), ("bv", [D]), ("bo", [D]),
                  ("bi", [FF]), ("bo2", [D]), ("ln1_g", [D]), ("ln1_b", [D]),
                  ("ln2_g", [D]), ("ln2_b", [D])):
        ap[n] = nc.dram_tensor(n, sh, F32, kind="ExternalInput").ap()
    ap["outT"] = nc.dram_tensor("outT", [D, T], F32, kind="ExternalOutput").ap()
    ap["og"] = nc.dram_tensor("og", [H, 65, NG], F32, kind="ExternalOutput").ap()
    # DRAM scratch for softmax-denominator broadcast round-trips
    ap["scr"] = nc.dram_tensor("scr", [24, HALF], BF16).ap()

    with tile.TileContext(nc) as tc:
        if repeat > 1:
            def body(i):
                with ExitStack() as c2:
                    build_body(nc, tc, ap, c2, sim_mode)
            tc.For_i_unrolled(0, repeat, 1, body, max_unroll=1)
        else:
            with ExitStack() as c2:
                build_body(nc, tc, ap, c2, sim_mode)
    nc.compile()
    return nc


# ---------------- host side ----------------

def shard_inputs(inputs):
    hs = np.asarray(inputs["hidden_states"], np.float32)
    am = np.asarray(inputs["attention_mask"], np.float32)
    shared = {}
    for n in ("Wq", "bq", "Wk", "bk", "Wv", "bv", "Wo", "bo", "ln1_g", "ln1_b",
              "Wi", "bi", "Wo2", "bo2", "ln2_g", "ln2_b"):
        shared[n] = np.ascontiguousarray(np.asarray(inputs[n], np.float32))
    for n in ("Wq", "Wk", "Wv", "Wo", "Wi", "Wo2"):
        shared[n] = shared[n].astype(ml_dtypes.bfloat16)
    in_maps = []
    for core in range(N_CORES):
        b, q0 = core // 4, (core % 4) * T
        xh = np.zeros((TH, D), np.float32)
        amh = np.zeros((TH,), np.float32)
        lo, hi = q0 - W, q0 + T + W
        slo, shi = max(lo, 0), min(hi, S)
        xh[slo - lo:shi - lo] = hs[b, slo:shi]
        amh[slo - lo:shi - lo] = am[b, slo:shi]
        mask = np.zeros((NJ, 128, 3 * W), np.float32)
        for j in range(-1, NCH + 1):
            cs = [c for c in (j - 1, j, j + 1) if 0 <= c < NCH]
            kpos = q0 + j * 128 + np.arange(128)[:, None]
            for i, c in enumerate(cs):
                qpos = q0 + c * 128 + np.arange(128)[None, :]
                valid = (np.abs(kpos - qpos) <= W) & (kpos >= NG) & (kpos >= 0) \
                    & (kpos < S)
                mask[j + 1, :, i * 128:(i + 1) * 128] = valid
        amg = np.ascontiguousarray(am[b, :NG])
        m = {"xT": np.ascontiguousarray(xh.T).astype(ml_dtypes.bfloat16),
             "xgT": np.ascontiguousarray(hs[b, :NG].T).astype(ml_dtypes.bfloat16),
             "am_halo": amh,
             "am_glob2": np.concatenate([amg, amg]),
             "mask_all": np.ascontiguousarray(
                 mask.transpose(1, 0, 2)).astype(ml_dtypes.bfloat16)}
        m.update(shared)
        in_maps.append(m)
    return in_maps


def _np_layernorm(x, g, b):
    mu = x.mean(-1, keepdims=True)
    var = ((x - mu) ** 2).mean(-1, keepdims=True)
    return (x - mu) / np.sqrt(var + EPS) * g + b


def _np_gelu(x):
    from scipy.special import erf
    return x * 0.5 * (1.0 + erf(x / np.sqrt(2.0)))


def host_tail(inputs, og_by_core, sim_mode=False):
    """Combine global-query flash partials; dense tail for the global rows."""
    hs = np.asarray(inputs["hidden_states"], np.float64)
    rows = np.zeros((B, NG, D))
    for b in range(B):
        o = sum(np.asarray(og_by_core[4 * b + c], np.float64) for c in range(4))
        gctx = o[:, :DH, :] / o[:, 64:65, :]          # [H, DH, NG]
        gctx = gctx.transpose(2, 0, 1).reshape(NG, D)  # feature index = h*64+d
        u = gctx @ np.asarray(inputs["Wo"], np.float64) \
            + np.asarray(inputs["bo"], np.float64) + hs[b, :NG]
        a = _np_layernorm(u, np.asarray(inputs["ln1_g"], np.float64),
                          np.asarray(inputs["ln1_b"], np.float64))
        inter = a @ np.asarray(inputs["Wi"], np.float64) \
            + np.asarray(inputs["bi"], np.float64)
        if not sim_mode:
            inter = _np_gelu(inter)
        u2 = inter @ np.asarray(inputs["Wo2"], np.float64) \
            + np.asarray(inputs["bo2"], np.float64) + a
        rows[b] = _np_layernorm(u2, np.asarray(inputs["ln2_g"], np.float64),
                                np.asarray(inputs["ln2_b"], np.float64))
    return rows.astype(np.float32)


def assemble(inputs, results, sim_mode=False):
    out = np.zeros((B, S, D), np.float32)
    for core in range(N_CORES):
        b, q0 = core // 4, (core % 4) * T
        out[b, q0:q0 + T] = np.asarray(results[core]["outT"]).T
    out[:, :NG] = host_tail(inputs, [results[c]["og"] for c in range(N_CORES)],
                            sim_mode)
    return out


def kernel(**inputs):
    from concourse import bass_utils
    if "nc" not in _nc_cache:
        _nc_cache["nc"] = build_nc()
    nc = _nc_cache["nc"]
    in_maps = shard_inputs(inputs)
    res = bass_utils.run_bass_kernel_spmd(nc, in_maps, core_ids=list(range(N_CORES)))
    return assemble(inputs, res.results)
